# revision 20
# baseline (speedup 1.0000x reference)
"""ChainKinematics Trainium2 kernel (8-core data-parallel).

Math per batch element b:
  T_curr_i = offsets[i] @ Rz(theta[b, i])
  abs_i = abs_{i-1} @ T_curr_i           (abs_{-1} = I)
  rel_i = reset_i ? T_curr_i : rel_{i-1} @ T_curr_i

Device mapping (per core, 8192 batch elements):
  State S holds A (4x4 per batch elem) as S[k*32+g, r*256+bw] = A[g*256+bw, r, k]
  (column k on partition blocks of 32, row r in free dim).
  Step: U = A @ O_i on TensorE via block-diag lhsT emitting m-blocks
  [u0, u1, u1, u0] (dup) + [u2, u3]; then the Rz mix on DVE as two
  full products PC = [c*u0 | c*u1], QS = [s*u1 | -s*u0] (the trig tile
  has partition blocks [c, c, s, -s]); GPSIMD adds PC+QS -> new cols 0,1;
  ScalarE copies u2,u3 -> new cols 2,3.  cos/sin computed on device via
  magic-number range reduction + ACT Sin LUT.
"""

import sys

sys.path.insert(0, "/opt/trn_rl_repo")

import numpy as np

N_BODIES = 32
BATCH = 65536
N_CORES = 8
BC = BATCH // N_CORES  # 8192 per core
G = 32  # batch groups (partition blocks)
BW = BC // G  # 256 batch per group
FH = 4 * BW  # 1024: free size of one chain-slot (r, bw)
MAGIC = float(1.5 * 2**23)
TWO_PI = float(2 * np.pi)
INV2PI = float(1.0 / TWO_PI)

_cache = {}


def _build_program(resets):
    """Build the Bass program. resets: sorted tuple of rel-restart bodies (>0)."""
    from concourse import bass, mybir, tile, bacc

    f32 = mybir.dt.float32
    f32r = mybir.dt.float32r

    split = resets[0] if resets else N_BODIES  # first dual body

    nc = bacc.Bacc(None, target_bir_lowering=False, debug=False)
    threp_d = nc.dram_tensor("threp", [128, BC], f32, kind="ExternalInput")
    wall_d = nc.dram_tensor("wall", [128, N_BODIES * 192], f32r, kind="ExternalInput")
    oabs_d = nc.dram_tensor("oabs", [N_BODIES, 128, FH], f32r, kind="ExternalOutput")
    orel_d = nc.dram_tensor(
        "orel", [N_BODIES - split, 128, FH], f32r, kind="ExternalOutput"
    )

    with tile.TileContext(nc) as tc:
        with (
            tc.tile_pool(name="wpool", bufs=1) as wpool,
            tc.tile_pool(name="trigpool", bufs=1) as trigpool,
            tc.tile_pool(name="cpool", bufs=1) as cpool,
        ):
            w_tile = wpool.tile([128, N_BODIES * 192], f32r)
            nc.sync.dma_start(w_tile[:], wall_d[:])
            trig = trigpool.tile([128, BC], f32)

            # per-partition constants: blocks [c, c, s, -s]
            m_b = cpool.tile([128, 1], f32)
            scl = cpool.tile([128, 1], f32)
            bias = cpool.tile([128, 1], f32)
            nc.vector.memset(m_b[0:64, :], 0.25)
            nc.vector.memset(m_b[64:128, :], 0.0)
            nc.vector.memset(scl[0:96, :], 1.0)
            nc.vector.memset(scl[96:128, :], -1.0)
            nc.vector.memset(bias[0:64, :], float(np.pi / 2))
            nc.vector.memset(bias[64:128, :], 0.0)

            # ---- trig phase (scratch freed afterwards) ----
            # body-major free layout: f = i*BW + bw. Computed in chunks so the
            # chain scan can start as soon as the first bodies' trig is ready.
            with tc.tile_pool(name="scratch", bufs=2) as sp:
                threp = trigpool.tile([128, BC], f32, tag="threp")
                nc.sync.dma_start(threp[:], threp_d[:])
                bounds = [0, 2 * BW, 8 * BW, BC]
                for lo, hi in zip(bounds[:-1], bounds[1:]):
                    sl = slice(lo, hi)
                    n = hi - lo
                    y1 = sp.tile([128, n], f32, tag="y")
                    nc.vector.tensor_scalar(
                        y1[:], threp[:, sl], INV2PI, m_b[:, 0:1],
                        mybir.AluOpType.mult, mybir.AluOpType.add,
                    )
                    y2 = sp.tile([128, n], f32, tag="y")
                    nc.vector.tensor_scalar(
                        y2[:], y1[:], MAGIC, None, mybir.AluOpType.add
                    )
                    y3 = sp.tile([128, n], f32, tag="y")
                    nc.vector.tensor_scalar(
                        y3[:], y2[:], MAGIC, None, mybir.AluOpType.subtract
                    )
                    y4 = sp.tile([128, n], f32, tag="y")
                    nc.vector.scalar_tensor_tensor(
                        y4[:], y3[:], -TWO_PI, threp[:, sl],
                        mybir.AluOpType.mult, mybir.AluOpType.add,
                    )
                    nc.scalar.activation(
                        trig[:, sl], y4[:], mybir.ActivationFunctionType.Sin,
                        bias=bias[:, 0:1], scale=scl[:, 0:1],
                    )

            # ---- state phase ----
            with (
                tc.tile_pool(name="spool", bufs=6) as spool,
                tc.tile_pool(name="idpool", bufs=1) as idpool,
                tc.tile_pool(name="mixpool", bufs=10) as mixpool,
                tc.tile_pool(name="u2pool", bufs=4, space=bass.MemorySpace.PSUM) as u2pool,
                tc.tile_pool(name="u23pool", bufs=4, space=bass.MemorySpace.PSUM) as u23pool,
            ):
                sid_f = idpool.tile([128, FH], f32)
                nc.vector.memset(sid_f[:], 0.0)
                for k in range(4):
                    nc.vector.memset(
                        sid_f[k * 32 : (k + 1) * 32, k * BW : (k + 1) * BW], 1.0
                    )
                sid = idpool.tile([128, FH], f32r)
                nc.vector.tensor_copy(sid[:], sid_f[:])

                s_prev = None
                for i in range(N_BODIES):
                    dual = i >= split
                    s_next = spool.tile([128, 2 * FH], f32r, tag="state")
                    slots = [0, 1] if dual else [0]
                    for slot in slots:
                        if i == 0 or (slot == 1 and i in resets):
                            rhs = sid[:]
                        elif slot == 1 and i == split:
                            # first dual body: rel restarts at split, so this
                            # branch is covered by the reset case above
                            rhs = sid[:]
                        else:
                            # rel before split equals abs (slot 0 of s_prev)
                            off = FH if (slot == 1 and i > split) else 0
                            rhs = s_prev[:, off : off + FH]
                        fo = slot * FH  # free offset in s_next
                        wd = w_tile[:, i * 192 : i * 192 + 128]
                        w2 = w_tile[:, i * 192 + 128 : i * 192 + 192]
                        # split single-chain bodies into two independent free
                        # sub-halves (r in {0,1} and r in {2,3}) to deepen
                        # the PE->DVE->POOL/ACT pipeline; dual bodies already
                        # have 2-way chain parallelism so keep ops full-width
                        SUB = 512
                        for sub in range(0, FH, SUB):
                            nr = SUB // BW  # r-values in this sub-slot
                            u2 = u2pool.tile([128, SUB], mybir.dt.float32, tag="u2")
                            u23 = u23pool.tile([64, SUB], mybir.dt.float32, tag="u23")
                            csz = min(512, SUB)
                            for ch in range(0, SUB, csz):
                                ms = slice(sub + ch, sub + ch + csz)
                                us = slice(ch, ch + csz)
                                nc.tensor.matmul(
                                    u2[:, us], wd, rhs[:, ms], start=True, stop=True
                                )
                                nc.tensor.matmul(
                                    u23[:, us], w2, rhs[:, ms], start=True, stop=True
                                )
                            tsl = slice(i * BW, (i + 1) * BW)
                            cc_b = (
                                trig[0:64, tsl]
                                .unsqueeze(1)
                                .broadcast_to([64, nr, BW])
                            )
                            sn_b = (
                                trig[64:128, tsl]
                                .unsqueeze(1)
                                .broadcast_to([64, nr, BW])
                            )
                            pc = mixpool.tile([64, SUB], f32, tag="pc")
                            qs = mixpool.tile([64, SUB], f32, tag="qs")
                            u2v_lo = u2[0:64, :].rearrange("p (r b) -> p r b", b=BW)
                            u2v_hi = u2[64:128, :].rearrange("p (r b) -> p r b", b=BW)
                            nc.vector.tensor_mul(
                                pc[:].rearrange("p (r b) -> p r b", b=BW), u2v_lo, cc_b
                            )
                            nc.vector.tensor_mul(
                                qs[:].rearrange("p (r b) -> p r b", b=BW), u2v_hi, sn_b
                            )
                            nc.gpsimd.tensor_add(
                                s_next[0:64, fo + sub : fo + sub + SUB], qs[:], pc[:]
                            )
                            nc.scalar.copy(
                                s_next[64:128, fo + sub : fo + sub + SUB], u23[:]
                            )
                        if slot == 0:
                            nc.sync.dma_start(oabs_d[i, :, :], s_next[:, 0:FH])
                        else:
                            nc.sync.dma_start(
                                orel_d[i - split, :, :], s_next[:, FH : 2 * FH]
                            )
                    s_prev = s_next

    nc.compile()
    return nc, split


def kernel(theta, offsets, reset_mask):
    theta = np.asarray(theta, dtype=np.float32)
    offsets = np.asarray(offsets, dtype=np.float32)
    reset_mask = np.asarray(reset_mask)
    assert theta.shape == (BATCH, N_BODIES)
    assert bool(reset_mask[0]), "chain must reset at body 0"
    resets = tuple(int(i) for i in np.flatnonzero(reset_mask) if i > 0)

    from concourse.bass_utils import run_bass_kernel_spmd
    import os

    key = resets
    if key not in _cache:
        _cache[key] = _build_program(resets)
    nc, split = _cache[key]

    # host-prepared weights: per body, lhsT blocks for [u0,u1,u1,u0] and [u2,u3]
    W_all = np.zeros((128, N_BODIES * 192), np.float32)
    gidx = np.arange(G)
    for i in range(N_BODIES):
        O = offsets[i]
        for k in range(4):
            for mb, j in enumerate([0, 1, 1, 0]):
                W_all[k * G + gidx, i * 192 + mb * G + gidx] = O[k, j]
            for mb, j in enumerate([2, 3]):
                W_all[k * G + gidx, i * 192 + 128 + mb * G + gidx] = O[k, j]

    # host-prepared theta: [128, BC] with partition blocks [c,c,s,-s] all equal
    # to theta in layout [g, (bw, i)]; value th[g*BW+bw, i] at (q*32+g, bw*32+i)
    in_maps = []
    for c in range(N_CORES):
        thc = theta[c * BC : (c + 1) * BC]  # [8192, 32]
        th_g = np.ascontiguousarray(
            thc.reshape(G, BW, N_BODIES).transpose(0, 2, 1).reshape(G, BW * N_BODIES)
        )  # [32, 8192]
        threp = np.tile(th_g, (4, 1))  # [128, 8192]
        in_maps.append({"threp": threp, "wall": W_all})

    out = run_bass_kernel_spmd(nc, in_maps, core_ids=list(range(N_CORES)))
    kernel.last_exec_ns = out.exec_time_ns
    kernel.last_results = out

    def decode(arr):
        # [nb, 128, FH] -> [nb, BC, 4, 4]: p=(k,g), f=(r,bw)
        nb = arr.shape[0]
        a = arr.reshape(nb, 4, G, 4, BW)  # i, k, g, r, bw
        return np.ascontiguousarray(
            a.transpose(0, 2, 4, 3, 1).reshape(nb, BC, 4, 4)
        )

    abs_full = np.empty((N_BODIES, BATCH, 4, 4), np.float32)
    rel_full = np.empty((N_BODIES, BATCH, 4, 4), np.float32)
    for c in range(N_CORES):
        res = out.results[c]
        bsl = slice(c * BC, (c + 1) * BC)
        abs_full[:, bsl] = decode(res["oabs"])
        rel_full[split:, bsl] = decode(res["orel"])
    rel_full[:split] = abs_full[:split]
    return abs_full, rel_full


kernel.last_exec_ns = None
kernel.last_results = None


# revision 21
# speedup vs baseline: 1.0692x; 1.0692x over previous
"""ChainKinematics Trainium2 kernel (8-core data-parallel).

Math per batch element b:
  T_curr_i = offsets[i] @ Rz(theta[b, i])
  abs_i = abs_{i-1} @ T_curr_i           (abs_{-1} = I)
  rel_i = reset_i ? T_curr_i : rel_{i-1} @ T_curr_i

Device mapping (per core, 8192 batch elements):
  State S holds A (4x4 per batch elem) as S[k*32+g, r*256+bw] = A[g*256+bw, r, k]
  (column k on partition blocks of 32, row r in free dim).
  Step: U = A @ O_i on TensorE via block-diag lhsT emitting m-blocks
  [u0, u1, u1, u0] (dup) + [u2, u3]; then the Rz mix on DVE as two
  full products PC = [c*u0 | c*u1], QS = [s*u1 | -s*u0] (the trig tile
  has partition blocks [c, c, s, -s]); GPSIMD adds PC+QS -> new cols 0,1;
  ScalarE copies u2,u3 -> new cols 2,3.  cos/sin computed on device via
  magic-number range reduction + ACT Sin LUT.
"""

import sys

sys.path.insert(0, "/opt/trn_rl_repo")

import numpy as np

N_BODIES = 32
BATCH = 65536
N_CORES = 8
BC = BATCH // N_CORES  # 8192 per core
G = 32  # batch groups (partition blocks)
BW = BC // G  # 256 batch per group
FH = 4 * BW  # 1024: free size of one chain-slot (r, bw)
MAGIC = float(1.5 * 2**23)
TWO_PI = float(2 * np.pi)
INV2PI = float(1.0 / TWO_PI)

_cache = {}


def _build_program(resets):
    """Build the Bass program. resets: sorted tuple of rel-restart bodies (>0)."""
    from concourse import bass, mybir, tile, bacc

    f32 = mybir.dt.float32
    f32r = mybir.dt.float32r

    split = resets[0] if resets else N_BODIES  # first dual body

    nc = bacc.Bacc(None, target_bir_lowering=False, debug=False)
    threp_d = nc.dram_tensor("threp", [128, BC], f32, kind="ExternalInput")
    wall_d = nc.dram_tensor("wall", [128, N_BODIES * 192], f32r, kind="ExternalInput")
    wsum_d = nc.dram_tensor("wsum", [128, 64], f32r, kind="ExternalInput")
    oabs_d = nc.dram_tensor("oabs", [N_BODIES, 128, FH], f32r, kind="ExternalOutput")
    orel_d = nc.dram_tensor(
        "orel", [N_BODIES - split, 128, FH], f32r, kind="ExternalOutput"
    )

    with tile.TileContext(nc) as tc:
        with (
            tc.tile_pool(name="wpool", bufs=1) as wpool,
            tc.tile_pool(name="trigpool", bufs=1) as trigpool,
            tc.tile_pool(name="cpool", bufs=1) as cpool,
        ):
            w_tile = wpool.tile([128, N_BODIES * 192], f32r)
            nc.sync.dma_start(w_tile[:], wall_d[:])
            wsum = wpool.tile([128, 64], f32r)
            nc.sync.dma_start(wsum[:], wsum_d[:])
            trig = trigpool.tile([128, BC], f32)

            # per-partition constants: blocks [c, c, s, -s]
            m_b = cpool.tile([128, 1], f32)
            scl = cpool.tile([128, 1], f32)
            bias = cpool.tile([128, 1], f32)
            nc.vector.memset(m_b[0:64, :], 0.25)
            nc.vector.memset(m_b[64:128, :], 0.0)
            nc.vector.memset(scl[0:96, :], 1.0)
            nc.vector.memset(scl[96:128, :], -1.0)
            nc.vector.memset(bias[0:64, :], float(np.pi / 2))
            nc.vector.memset(bias[64:128, :], 0.0)

            # ---- trig phase (scratch freed afterwards) ----
            # body-major free layout: f = i*BW + bw. Computed in chunks so the
            # chain scan can start as soon as the first bodies' trig is ready.
            with tc.tile_pool(name="scratch", bufs=2) as sp:
                threp = trigpool.tile([128, BC], f32, tag="threp")
                nc.sync.dma_start(threp[:], threp_d[:])
                bounds = [0, 2 * BW, 8 * BW, BC]
                for lo, hi in zip(bounds[:-1], bounds[1:]):
                    sl = slice(lo, hi)
                    n = hi - lo
                    y1 = sp.tile([128, n], f32, tag="y")
                    nc.vector.tensor_scalar(
                        y1[:], threp[:, sl], INV2PI, m_b[:, 0:1],
                        mybir.AluOpType.mult, mybir.AluOpType.add,
                    )
                    y2 = sp.tile([128, n], f32, tag="y")
                    nc.vector.tensor_scalar(
                        y2[:], y1[:], MAGIC, None, mybir.AluOpType.add
                    )
                    y3 = sp.tile([128, n], f32, tag="y")
                    nc.vector.tensor_scalar(
                        y3[:], y2[:], MAGIC, None, mybir.AluOpType.subtract
                    )
                    y4 = sp.tile([128, n], f32, tag="y")
                    nc.vector.scalar_tensor_tensor(
                        y4[:], y3[:], -TWO_PI, threp[:, sl],
                        mybir.AluOpType.mult, mybir.AluOpType.add,
                    )
                    nc.scalar.activation(
                        trig[:, sl], y4[:], mybir.ActivationFunctionType.Sin,
                        bias=bias[:, 0:1], scale=scl[:, 0:1],
                    )

            # ---- state phase ----
            with (
                tc.tile_pool(name="spool", bufs=6) as spool,
                tc.tile_pool(name="idpool", bufs=1) as idpool,
                tc.tile_pool(name="mixpool", bufs=10) as mixpool,
                tc.tile_pool(name="u2pool", bufs=3, space=bass.MemorySpace.PSUM) as u2pool,
                tc.tile_pool(name="u23pool", bufs=2, space=bass.MemorySpace.PSUM) as u23pool,
                tc.tile_pool(name="sumpool", bufs=2, space=bass.MemorySpace.PSUM) as sumpool,
            ):
                sid_f = idpool.tile([128, FH], f32)
                nc.vector.memset(sid_f[:], 0.0)
                for k in range(4):
                    nc.vector.memset(
                        sid_f[k * 32 : (k + 1) * 32, k * BW : (k + 1) * BW], 1.0
                    )
                sid = idpool.tile([128, FH], f32r)
                nc.vector.tensor_copy(sid[:], sid_f[:])

                s_prev = None
                for i in range(N_BODIES):
                    dual = i >= split
                    s_next = spool.tile([128, 2 * FH], f32r, tag="state")
                    slots = [0, 1] if dual else [0]
                    for slot in slots:
                        if i == 0 or (slot == 1 and i in resets):
                            rhs = sid[:]
                        elif slot == 1 and i == split:
                            # first dual body: rel restarts at split, so this
                            # branch is covered by the reset case above
                            rhs = sid[:]
                        else:
                            # rel before split equals abs (slot 0 of s_prev)
                            off = FH if (slot == 1 and i > split) else 0
                            rhs = s_prev[:, off : off + FH]
                        fo = slot * FH  # free offset in s_next
                        wd = w_tile[:, i * 192 : i * 192 + 128]
                        w2 = w_tile[:, i * 192 + 128 : i * 192 + 192]
                        # split single-chain bodies into two independent free
                        # sub-halves (r in {0,1} and r in {2,3}) to deepen
                        # the PE->DVE->POOL/ACT pipeline; dual bodies already
                        # have 2-way chain parallelism so keep ops full-width
                        SUB = 512
                        for sub in range(0, FH, SUB):
                            nr = SUB // BW  # r-values in this sub-slot
                            u2 = u2pool.tile([128, SUB], mybir.dt.float32, tag="u2")
                            u23 = u23pool.tile([64, SUB], mybir.dt.float32, tag="u23")
                            csz = min(512, SUB)
                            for ch in range(0, SUB, csz):
                                ms = slice(sub + ch, sub + ch + csz)
                                us = slice(ch, ch + csz)
                                nc.tensor.matmul(
                                    u2[:, us], wd, rhs[:, ms], start=True, stop=True
                                )
                                nc.tensor.matmul(
                                    u23[:, us], w2, rhs[:, ms], start=True, stop=True
                                )
                            tsl = slice(i * BW, (i + 1) * BW)
                            tb = (
                                trig[:, tsl]
                                .unsqueeze(1)
                                .broadcast_to([128, nr, BW])
                            )
                            pq = mixpool.tile([128, SUB], f32r, tag="pq")
                            nc.vector.tensor_mul(
                                pq[:].rearrange("p (r b) -> p r b", b=BW),
                                u2[:].rearrange("p (r b) -> p r b", b=BW),
                                tb,
                            )
                            c01 = sumpool.tile([64, SUB], mybir.dt.float32, tag="c01")
                            nc.tensor.matmul(
                                c01[:], wsum[:], pq[:], start=True, stop=True
                            )
                            nc.scalar.copy(
                                s_next[0:64, fo + sub : fo + sub + SUB], c01[:]
                            )
                            nc.scalar.copy(
                                s_next[64:128, fo + sub : fo + sub + SUB], u23[:]
                            )
                        if slot == 0:
                            nc.sync.dma_start(oabs_d[i, :, :], s_next[:, 0:FH])
                        else:
                            nc.sync.dma_start(
                                orel_d[i - split, :, :], s_next[:, FH : 2 * FH]
                            )
                    s_prev = s_next

    nc.compile()
    return nc, split


def kernel(theta, offsets, reset_mask):
    theta = np.asarray(theta, dtype=np.float32)
    offsets = np.asarray(offsets, dtype=np.float32)
    reset_mask = np.asarray(reset_mask)
    assert theta.shape == (BATCH, N_BODIES)
    assert bool(reset_mask[0]), "chain must reset at body 0"
    resets = tuple(int(i) for i in np.flatnonzero(reset_mask) if i > 0)

    from concourse.bass_utils import run_bass_kernel_spmd
    import os

    key = resets
    if key not in _cache:
        _cache[key] = _build_program(resets)
    nc, split = _cache[key]

    # block-sum lhsT: col0 = PQ0 + PQ2, col1 = PQ1 + PQ3
    W_sum = np.zeros((128, 64), np.float32)
    for q, j in [(0, 0), (2, 0), (1, 1), (3, 1)]:
        W_sum[q * G + np.arange(G), j * G + np.arange(G)] = 1.0
    # host-prepared weights: per body, lhsT blocks for [u0,u1,u1,u0] and [u2,u3]
    W_all = np.zeros((128, N_BODIES * 192), np.float32)
    gidx = np.arange(G)
    for i in range(N_BODIES):
        O = offsets[i]
        for k in range(4):
            for mb, j in enumerate([0, 1, 1, 0]):
                W_all[k * G + gidx, i * 192 + mb * G + gidx] = O[k, j]
            for mb, j in enumerate([2, 3]):
                W_all[k * G + gidx, i * 192 + 128 + mb * G + gidx] = O[k, j]

    # host-prepared theta: [128, BC] with partition blocks [c,c,s,-s] all equal
    # to theta in layout [g, (bw, i)]; value th[g*BW+bw, i] at (q*32+g, bw*32+i)
    in_maps = []
    for c in range(N_CORES):
        thc = theta[c * BC : (c + 1) * BC]  # [8192, 32]
        th_g = np.ascontiguousarray(
            thc.reshape(G, BW, N_BODIES).transpose(0, 2, 1).reshape(G, BW * N_BODIES)
        )  # [32, 8192]
        threp = np.tile(th_g, (4, 1))  # [128, 8192]
        in_maps.append({"threp": threp, "wall": W_all, "wsum": W_sum})

    out = run_bass_kernel_spmd(nc, in_maps, core_ids=list(range(N_CORES)))
    kernel.last_exec_ns = out.exec_time_ns
    kernel.last_results = out

    def decode(arr):
        # [nb, 128, FH] -> [nb, BC, 4, 4]: p=(k,g), f=(r,bw)
        nb = arr.shape[0]
        a = arr.reshape(nb, 4, G, 4, BW)  # i, k, g, r, bw
        return np.ascontiguousarray(
            a.transpose(0, 2, 4, 3, 1).reshape(nb, BC, 4, 4)
        )

    abs_full = np.empty((N_BODIES, BATCH, 4, 4), np.float32)
    rel_full = np.empty((N_BODIES, BATCH, 4, 4), np.float32)
    for c in range(N_CORES):
        res = out.results[c]
        bsl = slice(c * BC, (c + 1) * BC)
        abs_full[:, bsl] = decode(res["oabs"])
        rel_full[split:, bsl] = decode(res["orel"])
    rel_full[:split] = abs_full[:split]
    return abs_full, rel_full


kernel.last_exec_ns = None
kernel.last_results = None


# revision 23
# speedup vs baseline: 1.1907x; 1.1137x over previous
"""ChainKinematics Trainium2 kernel (8-core data-parallel).

Math per batch element b:
  T_curr_i = offsets[i] @ Rz(theta[b, i])
  abs_i = abs_{i-1} @ T_curr_i           (abs_{-1} = I)
  rel_i = reset_i ? T_curr_i : rel_{i-1} @ T_curr_i

Device mapping (per core, 8192 batch elements):
  State S holds A (4x4 per batch elem) as S[k*32+g, r*256+bw] = A[g*256+bw, r, k]
  (column k on partition blocks of 32, row r in free dim).
  Step: U = A @ O_i on TensorE via block-diag lhsT emitting m-blocks
  [u0, u1, u1, u0] (dup) + [u2, u3]; then the Rz mix on DVE as two
  full products PC = [c*u0 | c*u1], QS = [s*u1 | -s*u0] (the trig tile
  has partition blocks [c, c, s, -s]); GPSIMD adds PC+QS -> new cols 0,1;
  ScalarE copies u2,u3 -> new cols 2,3.  cos/sin computed on device via
  magic-number range reduction + ACT Sin LUT.
"""

import sys

sys.path.insert(0, "/opt/trn_rl_repo")

import numpy as np

N_BODIES = 32
BATCH = 65536
N_CORES = 8
BC = BATCH // N_CORES  # 8192 per core
G = 32  # batch groups (partition blocks)
BW = BC // G  # 256 batch per group
FH = 4 * BW  # 1024: free size of one chain-slot (r, bw)
MAGIC = float(1.5 * 2**23)
TWO_PI = float(2 * np.pi)
INV2PI = float(1.0 / TWO_PI)

_cache = {}


def _build_program(resets):
    """Build the Bass program. resets: sorted tuple of rel-restart bodies (>0)."""
    from concourse import bass, mybir, tile, bacc

    f32 = mybir.dt.float32
    f32r = mybir.dt.float32r

    split = resets[0] if resets else N_BODIES  # first dual body

    nc = bacc.Bacc(None, target_bir_lowering=False, debug=False)
    threp_d = nc.dram_tensor("threp", [128, BC], f32, kind="ExternalInput")
    wall_d = nc.dram_tensor("wall", [128, N_BODIES * 192], f32r, kind="ExternalInput")
    wsum_d = nc.dram_tensor("wsum", [128, 64], f32r, kind="ExternalInput")
    oabs_d = nc.dram_tensor("oabs", [N_BODIES, 128, FH], f32r, kind="ExternalOutput")
    orel_d = nc.dram_tensor(
        "orel", [N_BODIES - split, 128, FH], f32r, kind="ExternalOutput"
    )

    with tile.TileContext(nc) as tc:
        with (
            tc.tile_pool(name="wpool", bufs=1) as wpool,
            tc.tile_pool(name="trigpool", bufs=1) as trigpool,
            tc.tile_pool(name="cpool", bufs=1) as cpool,
        ):
            w_tile = wpool.tile([128, N_BODIES * 192], f32r)
            nc.sync.dma_start(w_tile[:], wall_d[:])
            wsum = wpool.tile([128, 64], f32r)
            nc.sync.dma_start(wsum[:], wsum_d[:])
            trig = trigpool.tile([128, BC], f32)

            # per-partition constants: blocks [c, c, s, -s]
            m_b = cpool.tile([128, 1], f32)
            scl = cpool.tile([128, 1], f32)
            bias = cpool.tile([128, 1], f32)
            nc.vector.memset(m_b[0:64, :], 0.25)
            nc.vector.memset(m_b[64:128, :], 0.0)
            nc.vector.memset(scl[0:96, :], 1.0)
            nc.vector.memset(scl[96:128, :], -1.0)
            nc.vector.memset(bias[0:64, :], float(np.pi / 2))
            nc.vector.memset(bias[64:128, :], 0.0)

            # ---- trig phase (scratch freed afterwards) ----
            # body-major free layout: f = i*BW + bw. Computed in chunks so the
            # chain scan can start as soon as the first bodies' trig is ready.
            with tc.tile_pool(name="scratch", bufs=2) as sp:
                threp = trigpool.tile([128, BC], f32, tag="threp")
                nc.sync.dma_start(threp[:], threp_d[:])
                bounds = [0, 2 * BW, 8 * BW, BC]
                for lo, hi in zip(bounds[:-1], bounds[1:]):
                    sl = slice(lo, hi)
                    n = hi - lo
                    y1 = sp.tile([128, n], f32, tag="y")
                    nc.vector.tensor_scalar(
                        y1[:], threp[:, sl], INV2PI, m_b[:, 0:1],
                        mybir.AluOpType.mult, mybir.AluOpType.add,
                    )
                    y2 = sp.tile([128, n], f32, tag="y")
                    nc.vector.tensor_scalar(
                        y2[:], y1[:], MAGIC, None, mybir.AluOpType.add
                    )
                    y3 = sp.tile([128, n], f32, tag="y")
                    nc.vector.tensor_scalar(
                        y3[:], y2[:], MAGIC, None, mybir.AluOpType.subtract
                    )
                    y4 = sp.tile([128, n], f32, tag="y")
                    nc.vector.scalar_tensor_tensor(
                        y4[:], y3[:], -TWO_PI, threp[:, sl],
                        mybir.AluOpType.mult, mybir.AluOpType.add,
                    )
                    nc.scalar.activation(
                        trig[:, sl], y4[:], mybir.ActivationFunctionType.Sin,
                        bias=bias[:, 0:1], scale=scl[:, 0:1],
                    )

            # ---- state phase ----
            with (
                tc.tile_pool(name="spool", bufs=6) as spool,
                tc.tile_pool(name="idpool", bufs=1) as idpool,
                tc.tile_pool(name="mixpool", bufs=10) as mixpool,
                tc.tile_pool(name="u2pool", bufs=3, space=bass.MemorySpace.PSUM) as u2pool,
                tc.tile_pool(name="u23pool", bufs=2, space=bass.MemorySpace.PSUM) as u23pool,
                tc.tile_pool(name="sumpool", bufs=2, space=bass.MemorySpace.PSUM) as sumpool,
            ):
                sid_f = idpool.tile([128, FH], f32)
                nc.vector.memset(sid_f[:], 0.0)
                for k in range(4):
                    nc.vector.memset(
                        sid_f[k * 32 : (k + 1) * 32, k * BW : (k + 1) * BW], 1.0
                    )
                sid = idpool.tile([128, FH], f32r)
                nc.vector.tensor_copy(sid[:], sid_f[:])

                s_prev = None
                nsub = [0]
                for i in range(N_BODIES):
                    dual = i >= split
                    s_next = spool.tile([128, 2 * FH], f32r, tag="state")
                    slots = [0, 1] if dual else [0]
                    for slot in slots:
                        if i == 0 or (slot == 1 and i in resets):
                            rhs = sid[:]
                        elif slot == 1 and i == split:
                            # first dual body: rel restarts at split, so this
                            # branch is covered by the reset case above
                            rhs = sid[:]
                        else:
                            # rel before split equals abs (slot 0 of s_prev)
                            off = FH if (slot == 1 and i > split) else 0
                            rhs = s_prev[:, off : off + FH]
                        fo = slot * FH  # free offset in s_next
                        wd = w_tile[:, i * 192 : i * 192 + 128]
                        w2 = w_tile[:, i * 192 + 128 : i * 192 + 192]
                        # split single-chain bodies into two independent free
                        # sub-halves (r in {0,1} and r in {2,3}) to deepen
                        # the PE->DVE->POOL/ACT pipeline; dual bodies already
                        # have 2-way chain parallelism so keep ops full-width
                        SUB = 512
                        for sub in range(0, FH, SUB):
                            nr = SUB // BW  # r-values in this sub-slot
                            u2 = u2pool.tile([128, SUB], mybir.dt.float32, tag="u2")
                            u23 = u23pool.tile([64, SUB], mybir.dt.float32, tag="u23")
                            csz = min(512, SUB)
                            for ch in range(0, SUB, csz):
                                ms = slice(sub + ch, sub + ch + csz)
                                us = slice(ch, ch + csz)
                                nc.tensor.matmul(
                                    u2[:, us], wd, rhs[:, ms], start=True, stop=True
                                )
                                nc.tensor.matmul(
                                    u23[:, us], w2, rhs[:, ms], start=True, stop=True
                                )
                            tsl = slice(i * BW, (i + 1) * BW)
                            tb = (
                                trig[:, tsl]
                                .unsqueeze(1)
                                .broadcast_to([128, nr, BW])
                            )
                            pq = mixpool.tile([128, SUB], f32r, tag="pq")
                            nc.vector.tensor_mul(
                                pq[:].rearrange("p (r b) -> p r b", b=BW),
                                u2[:].rearrange("p (r b) -> p r b", b=BW),
                                tb,
                            )
                            c01 = sumpool.tile([64, SUB], mybir.dt.float32, tag="c01")
                            nc.tensor.matmul(
                                c01[:], wsum[:], pq[:], start=True, stop=True
                            )
                            nsub[0] += 1
                            if nsub[0] % 3 == 0:
                                nc.vector.tensor_copy(
                                    s_next[0:64, fo + sub : fo + sub + SUB], c01[:]
                                )
                            else:
                                nc.scalar.copy(
                                    s_next[0:64, fo + sub : fo + sub + SUB], c01[:]
                                )
                            nc.scalar.copy(
                                s_next[64:128, fo + sub : fo + sub + SUB], u23[:]
                            )
                        if slot == 0:
                            nc.sync.dma_start(oabs_d[i, :, :], s_next[:, 0:FH])
                        else:
                            nc.sync.dma_start(
                                orel_d[i - split, :, :], s_next[:, FH : 2 * FH]
                            )
                    s_prev = s_next

    nc.compile()
    return nc, split


def kernel(theta, offsets, reset_mask):
    theta = np.asarray(theta, dtype=np.float32)
    offsets = np.asarray(offsets, dtype=np.float32)
    reset_mask = np.asarray(reset_mask)
    assert theta.shape == (BATCH, N_BODIES)
    assert bool(reset_mask[0]), "chain must reset at body 0"
    resets = tuple(int(i) for i in np.flatnonzero(reset_mask) if i > 0)

    from concourse.bass_utils import run_bass_kernel_spmd
    import os

    key = resets
    if key not in _cache:
        _cache[key] = _build_program(resets)
    nc, split = _cache[key]

    # block-sum lhsT: col0 = PQ0 + PQ2, col1 = PQ1 + PQ3
    W_sum = np.zeros((128, 64), np.float32)
    for q, j in [(0, 0), (2, 0), (1, 1), (3, 1)]:
        W_sum[q * G + np.arange(G), j * G + np.arange(G)] = 1.0
    # host-prepared weights: per body, lhsT blocks for [u0,u1,u1,u0] and [u2,u3]
    W_all = np.zeros((128, N_BODIES * 192), np.float32)
    gidx = np.arange(G)
    for i in range(N_BODIES):
        O = offsets[i]
        for k in range(4):
            for mb, j in enumerate([0, 1, 1, 0]):
                W_all[k * G + gidx, i * 192 + mb * G + gidx] = O[k, j]
            for mb, j in enumerate([2, 3]):
                W_all[k * G + gidx, i * 192 + 128 + mb * G + gidx] = O[k, j]

    # host-prepared theta: [128, BC] with partition blocks [c,c,s,-s] all equal
    # to theta in layout [g, (bw, i)]; value th[g*BW+bw, i] at (q*32+g, bw*32+i)
    in_maps = []
    for c in range(N_CORES):
        thc = theta[c * BC : (c + 1) * BC]  # [8192, 32]
        th_g = np.ascontiguousarray(
            thc.reshape(G, BW, N_BODIES).transpose(0, 2, 1).reshape(G, BW * N_BODIES)
        )  # [32, 8192]
        threp = np.tile(th_g, (4, 1))  # [128, 8192]
        in_maps.append({"threp": threp, "wall": W_all, "wsum": W_sum})

    out = run_bass_kernel_spmd(nc, in_maps, core_ids=list(range(N_CORES)))
    kernel.last_exec_ns = out.exec_time_ns
    kernel.last_results = out

    def decode(arr):
        # [nb, 128, FH] -> [nb, BC, 4, 4]: p=(k,g), f=(r,bw)
        nb = arr.shape[0]
        a = arr.reshape(nb, 4, G, 4, BW)  # i, k, g, r, bw
        return np.ascontiguousarray(
            a.transpose(0, 2, 4, 3, 1).reshape(nb, BC, 4, 4)
        )

    abs_full = np.empty((N_BODIES, BATCH, 4, 4), np.float32)
    rel_full = np.empty((N_BODIES, BATCH, 4, 4), np.float32)
    for c in range(N_CORES):
        res = out.results[c]
        bsl = slice(c * BC, (c + 1) * BC)
        abs_full[:, bsl] = decode(res["oabs"])
        rel_full[split:, bsl] = decode(res["orel"])
    rel_full[:split] = abs_full[:split]
    return abs_full, rel_full


kernel.last_exec_ns = None
kernel.last_results = None


# revision 24
# speedup vs baseline: 1.1995x; 1.0074x over previous
"""ChainKinematics Trainium2 kernel (8-core data-parallel).

Math per batch element b:
  T_curr_i = offsets[i] @ Rz(theta[b, i])
  abs_i = abs_{i-1} @ T_curr_i           (abs_{-1} = I)
  rel_i = reset_i ? T_curr_i : rel_{i-1} @ T_curr_i

Device mapping (per core, 8192 batch elements):
  State S holds A (4x4 per batch elem) as S[k*32+g, r*256+bw] = A[g*256+bw, r, k]
  (column k on partition blocks of 32, row r in free dim).
  Step: U = A @ O_i on TensorE via block-diag lhsT emitting m-blocks
  [u0, u1, u1, u0] (dup) + [u2, u3]; then the Rz mix on DVE as two
  full products PC = [c*u0 | c*u1], QS = [s*u1 | -s*u0] (the trig tile
  has partition blocks [c, c, s, -s]); GPSIMD adds PC+QS -> new cols 0,1;
  ScalarE copies u2,u3 -> new cols 2,3.  cos/sin computed on device via
  magic-number range reduction + ACT Sin LUT.
"""

import sys

sys.path.insert(0, "/opt/trn_rl_repo")

import numpy as np

N_BODIES = 32
BATCH = 65536
N_CORES = 8
BC = BATCH // N_CORES  # 8192 per core
G = 32  # batch groups (partition blocks)
BW = BC // G  # 256 batch per group
FH = 4 * BW  # 1024: free size of one chain-slot (r, bw)
MAGIC = float(1.5 * 2**23)
TWO_PI = float(2 * np.pi)
INV2PI = float(1.0 / TWO_PI)

_cache = {}


def _build_program(resets):
    """Build the Bass program. resets: sorted tuple of rel-restart bodies (>0)."""
    from concourse import bass, mybir, tile, bacc

    f32 = mybir.dt.float32
    f32r = mybir.dt.float32r

    split = resets[0] if resets else N_BODIES  # first dual body

    nc = bacc.Bacc(None, target_bir_lowering=False, debug=False)
    threp_d = nc.dram_tensor("threp", [128, BC], f32, kind="ExternalInput")
    wall_d = nc.dram_tensor("wall", [128, N_BODIES * 192], f32r, kind="ExternalInput")
    wsum_d = nc.dram_tensor("wsum", [128, 64], f32r, kind="ExternalInput")
    oabs_d = nc.dram_tensor("oabs", [N_BODIES, 128, FH], f32r, kind="ExternalOutput")
    orel_d = nc.dram_tensor(
        "orel", [N_BODIES - split, 128, FH], f32r, kind="ExternalOutput"
    )

    with tile.TileContext(nc) as tc:
        with (
            tc.tile_pool(name="wpool", bufs=1) as wpool,
            tc.tile_pool(name="trigpool", bufs=1) as trigpool,
            tc.tile_pool(name="cpool", bufs=1) as cpool,
        ):
            w_tile = wpool.tile([128, N_BODIES * 192], f32r)
            nc.sync.dma_start(w_tile[:], wall_d[:])
            wsum = wpool.tile([128, 64], f32r)
            nc.sync.dma_start(wsum[:], wsum_d[:])
            trig = trigpool.tile([128, BC], f32)

            # per-partition constants: blocks [c, c, s, -s]
            m_b = cpool.tile([128, 1], f32)
            scl = cpool.tile([128, 1], f32)
            bias = cpool.tile([128, 1], f32)
            nc.vector.memset(m_b[0:64, :], 0.25)
            nc.vector.memset(m_b[64:128, :], 0.0)
            nc.vector.memset(scl[0:96, :], 1.0)
            nc.vector.memset(scl[96:128, :], -1.0)
            nc.vector.memset(bias[0:64, :], float(np.pi / 2))
            nc.vector.memset(bias[64:128, :], 0.0)

            # ---- trig phase (scratch freed afterwards) ----
            # body-major free layout: f = i*BW + bw. Computed in chunks so the
            # chain scan can start as soon as the first bodies' trig is ready.
            with tc.tile_pool(name="scratch", bufs=2) as sp:
                threp = trigpool.tile([128, BC], f32, tag="threp")
                nc.sync.dma_start(threp[:], threp_d[:])
                bounds = [0, 2 * BW, 8 * BW, BC]
                for lo, hi in zip(bounds[:-1], bounds[1:]):
                    sl = slice(lo, hi)
                    n = hi - lo
                    y1 = sp.tile([128, n], f32, tag="y")
                    nc.vector.tensor_scalar(
                        y1[:], threp[:, sl], INV2PI, m_b[:, 0:1],
                        mybir.AluOpType.mult, mybir.AluOpType.add,
                    )
                    y2 = sp.tile([128, n], f32, tag="y")
                    nc.vector.tensor_scalar(
                        y2[:], y1[:], MAGIC, None, mybir.AluOpType.add
                    )
                    y3 = sp.tile([128, n], f32, tag="y")
                    nc.vector.tensor_scalar(
                        y3[:], y2[:], MAGIC, None, mybir.AluOpType.subtract
                    )
                    y4 = sp.tile([128, n], f32, tag="y")
                    nc.vector.scalar_tensor_tensor(
                        y4[:], y3[:], -TWO_PI, threp[:, sl],
                        mybir.AluOpType.mult, mybir.AluOpType.add,
                    )
                    nc.scalar.activation(
                        trig[:, sl], y4[:], mybir.ActivationFunctionType.Sin,
                        bias=bias[:, 0:1], scale=scl[:, 0:1],
                    )

            # ---- state phase ----
            with (
                tc.tile_pool(name="spool", bufs=6) as spool,
                tc.tile_pool(name="idpool", bufs=1) as idpool,
                tc.tile_pool(name="mixpool", bufs=10) as mixpool,
                tc.tile_pool(name="u2pool", bufs=3, space=bass.MemorySpace.PSUM) as u2pool,
                tc.tile_pool(name="u23pool", bufs=2, space=bass.MemorySpace.PSUM) as u23pool,
                tc.tile_pool(name="sumpool", bufs=2, space=bass.MemorySpace.PSUM) as sumpool,
            ):
                sid_f = idpool.tile([128, FH], f32)
                nc.vector.memset(sid_f[:], 0.0)
                for k in range(4):
                    nc.vector.memset(
                        sid_f[k * 32 : (k + 1) * 32, k * BW : (k + 1) * BW], 1.0
                    )
                sid = idpool.tile([128, FH], f32r)
                nc.vector.tensor_copy(sid[:], sid_f[:])

                s_prev = None
                nsub = [0]
                for i in range(N_BODIES):
                    dual = i >= split
                    s_next = spool.tile([128, 2 * FH], f32r, tag="state")
                    slots = [0, 1] if dual else [0]
                    for slot in slots:
                        if i == 0 or (slot == 1 and i in resets):
                            rhs = sid[:]
                        elif slot == 1 and i == split:
                            # first dual body: rel restarts at split, so this
                            # branch is covered by the reset case above
                            rhs = sid[:]
                        else:
                            # rel before split equals abs (slot 0 of s_prev)
                            off = FH if (slot == 1 and i > split) else 0
                            rhs = s_prev[:, off : off + FH]
                        fo = slot * FH  # free offset in s_next
                        wd = w_tile[:, i * 192 : i * 192 + 128]
                        w2 = w_tile[:, i * 192 + 128 : i * 192 + 192]
                        # split single-chain bodies into two independent free
                        # sub-halves (r in {0,1} and r in {2,3}) to deepen
                        # the PE->DVE->POOL/ACT pipeline; dual bodies already
                        # have 2-way chain parallelism so keep ops full-width
                        SUB = 512
                        for sub in range(0, FH, SUB):
                            nr = SUB // BW  # r-values in this sub-slot
                            u2 = u2pool.tile([128, SUB], mybir.dt.float32, tag="u2")
                            u23 = u23pool.tile([64, SUB], mybir.dt.float32, tag="u23")
                            csz = min(512, SUB)
                            for ch in range(0, SUB, csz):
                                ms = slice(sub + ch, sub + ch + csz)
                                us = slice(ch, ch + csz)
                                nc.tensor.matmul(
                                    u2[:, us], wd, rhs[:, ms], start=True, stop=True
                                )
                                nc.tensor.matmul(
                                    u23[:, us], w2, rhs[:, ms], start=True, stop=True
                                )
                            tsl = slice(i * BW, (i + 1) * BW)
                            tb = (
                                trig[:, tsl]
                                .unsqueeze(1)
                                .broadcast_to([128, nr, BW])
                            )
                            pq = mixpool.tile([128, SUB], f32r, tag="pq")
                            nc.vector.tensor_mul(
                                pq[:].rearrange("p (r b) -> p r b", b=BW),
                                u2[:].rearrange("p (r b) -> p r b", b=BW),
                                tb,
                            )
                            c01 = sumpool.tile([64, SUB], mybir.dt.float32, tag="c01")
                            nc.tensor.matmul(
                                c01[:], wsum[:], pq[:], start=True, stop=True
                            )
                            nsub[0] += 1
                            if nsub[0] % 2 == 0:
                                nc.vector.tensor_copy(
                                    s_next[0:64, fo + sub : fo + sub + SUB], c01[:]
                                )
                            else:
                                nc.scalar.copy(
                                    s_next[0:64, fo + sub : fo + sub + SUB], c01[:]
                                )
                            nc.scalar.copy(
                                s_next[64:128, fo + sub : fo + sub + SUB], u23[:]
                            )
                        if slot == 0:
                            nc.sync.dma_start(oabs_d[i, :, :], s_next[:, 0:FH])
                        else:
                            nc.sync.dma_start(
                                orel_d[i - split, :, :], s_next[:, FH : 2 * FH]
                            )
                    s_prev = s_next

    nc.compile()
    return nc, split


def kernel(theta, offsets, reset_mask):
    theta = np.asarray(theta, dtype=np.float32)
    offsets = np.asarray(offsets, dtype=np.float32)
    reset_mask = np.asarray(reset_mask)
    assert theta.shape == (BATCH, N_BODIES)
    assert bool(reset_mask[0]), "chain must reset at body 0"
    resets = tuple(int(i) for i in np.flatnonzero(reset_mask) if i > 0)

    from concourse.bass_utils import run_bass_kernel_spmd
    import os

    key = resets
    if key not in _cache:
        _cache[key] = _build_program(resets)
    nc, split = _cache[key]

    # block-sum lhsT: col0 = PQ0 + PQ2, col1 = PQ1 + PQ3
    W_sum = np.zeros((128, 64), np.float32)
    for q, j in [(0, 0), (2, 0), (1, 1), (3, 1)]:
        W_sum[q * G + np.arange(G), j * G + np.arange(G)] = 1.0
    # host-prepared weights: per body, lhsT blocks for [u0,u1,u1,u0] and [u2,u3]
    W_all = np.zeros((128, N_BODIES * 192), np.float32)
    gidx = np.arange(G)
    for i in range(N_BODIES):
        O = offsets[i]
        for k in range(4):
            for mb, j in enumerate([0, 1, 1, 0]):
                W_all[k * G + gidx, i * 192 + mb * G + gidx] = O[k, j]
            for mb, j in enumerate([2, 3]):
                W_all[k * G + gidx, i * 192 + 128 + mb * G + gidx] = O[k, j]

    # host-prepared theta: [128, BC] with partition blocks [c,c,s,-s] all equal
    # to theta in layout [g, (bw, i)]; value th[g*BW+bw, i] at (q*32+g, bw*32+i)
    in_maps = []
    for c in range(N_CORES):
        thc = theta[c * BC : (c + 1) * BC]  # [8192, 32]
        th_g = np.ascontiguousarray(
            thc.reshape(G, BW, N_BODIES).transpose(0, 2, 1).reshape(G, BW * N_BODIES)
        )  # [32, 8192]
        threp = np.tile(th_g, (4, 1))  # [128, 8192]
        in_maps.append({"threp": threp, "wall": W_all, "wsum": W_sum})

    out = run_bass_kernel_spmd(nc, in_maps, core_ids=list(range(N_CORES)))
    kernel.last_exec_ns = out.exec_time_ns
    kernel.last_results = out

    def decode(arr):
        # [nb, 128, FH] -> [nb, BC, 4, 4]: p=(k,g), f=(r,bw)
        nb = arr.shape[0]
        a = arr.reshape(nb, 4, G, 4, BW)  # i, k, g, r, bw
        return np.ascontiguousarray(
            a.transpose(0, 2, 4, 3, 1).reshape(nb, BC, 4, 4)
        )

    abs_full = np.empty((N_BODIES, BATCH, 4, 4), np.float32)
    rel_full = np.empty((N_BODIES, BATCH, 4, 4), np.float32)
    for c in range(N_CORES):
        res = out.results[c]
        bsl = slice(c * BC, (c + 1) * BC)
        abs_full[:, bsl] = decode(res["oabs"])
        rel_full[split:, bsl] = decode(res["orel"])
    rel_full[:split] = abs_full[:split]
    return abs_full, rel_full


kernel.last_exec_ns = None
kernel.last_results = None
